# revision 10
# baseline (speedup 1.0000x reference)
"""Trainium2 Bass kernel for nn_GCNNetwork (2-branch GCN + MLP head), 8 NeuronCores.

Strategy
--------
Each of the 8 cores owns a block of 512 destination nodes (dst block k =
nodes [512k, 512k+512)).  On the host we only do *index* preprocessing and a
pure permutation of the edge-weight values:

  - append self-loop edges (w=1), assign each edge to the core owning its
    destination, sort each core's edges by (src row, dst col), and lay the
    weight values out in a padded per-source-row slot array  w_perm[4096, 36]
    together with an int16 slot->dst-column index array (-1 = unused slot and
    duplicate-continuation slots) plus duplicate-fold masks.

On device, each core:
  - folds duplicate (src,dst) edges with shifted adds (all arithmetic on
    device), then builds its dense fp16 adjacency block A[4096, 512] with 32
    `local_scatter` ops (per-partition indexed scatter, auto-zeroing),
  - computes degrees as ones^T @ A (PSUM f32), dinv = 1/sqrt(deg),
    AllGathers dinv,
  - runs every GCN conv as dense matmuls:  out^T = X^T @ A  with
    X = (dinv * h) @ W, normalization folded into cheap row/col scalings,
  - AllGathers the dinv-scaled 200-wide activations between convs; linear
    layers and the node-mean stay sharded; one final AllReduce combines the
    branch means, and the small MLP head runs redundantly on every core in
    float32r.

All-zero biases + relu make each branch positively homogeneous, so branch
inputs are scaled by 2^20 (topo) / 2^14 (traffic) to keep fp16 activations
out of the subnormal range; the scale is removed exactly (power of two) when
the node-mean is taken.
"""

import os
import sys

import numpy as np

for _p in ("/opt/trn_rl_repo", "/root/.axon_site/_ro/trn_rl_repo"):
    if os.path.isdir(_p) and _p not in sys.path:
        sys.path.append(_p)

import concourse.bacc as bacc
import concourse.bass as bass
import concourse.mybir as mybir
import concourse.tile as tile
from concourse.bass_utils import run_bass_kernel_spmd

N = 4096          # nodes
NB = 512          # dst nodes per core
NCORES = 8
NT = 32           # src tiles of 128
F = 200           # true feature width
FP = 256          # padded feature width
SLOTS = 32        # max edges (src row -> this core's dst block), asserted
SLOTP = 36        # slot array width (shift padding, even)
S_TOPO = float(2 ** 20)
S_TRAF = float(2 ** 14)

F16 = mybir.dt.float16
F32 = mybir.dt.float32
F32R = mybir.dt.float32r
I16 = mybir.dt.int16
AX = mybir.AxisListType.X
ALU = mybir.AluOpType
ACTF = mybir.ActivationFunctionType


# --------------------------------------------------------------------------
# Host-side: index preprocessing + pure value permutation (no arithmetic).
# --------------------------------------------------------------------------

def _prep_branch(edge_index, edge_weight):
    """Shard edges by dst block; per core build padded slot layout.

    Returns list (per core) of dicts: wperm f32 [N, SLOTP], idx int16
    [N, SLOTP], m1/m2/m3 fp16 [N, SLOTP].
    """
    row = np.asarray(edge_index[0]).astype(np.int64)
    col = np.asarray(edge_index[1]).astype(np.int64)
    w = np.asarray(edge_weight).astype(np.float32)
    ar = np.arange(N, dtype=np.int64)
    row = np.concatenate([row, ar])
    col = np.concatenate([col, ar])
    w = np.concatenate([w, np.ones(N, np.float32)])

    out = []
    for k in range(NCORES):
        sel = (col >> 9) == k
        r = row[sel]
        c = col[sel] - NB * k
        ww = w[sel]
        order = np.lexsort((c, r))
        r, c, ww = r[order], c[order], ww[order]
        n = len(r)

        new_row = np.empty(n, bool)
        new_row[0] = True
        new_row[1:] = r[1:] != r[:-1]
        first_of_row = np.maximum.accumulate(np.where(new_row, np.arange(n), 0))
        slot = np.arange(n) - first_of_row
        assert slot.max() < SLOTS, f"slot overflow: {slot.max()}"

        dup = np.zeros(n, bool)
        dup[1:] = (r[1:] == r[:-1]) & (c[1:] == c[:-1])
        d1 = np.zeros(n, bool)
        d2 = np.zeros(n, bool)
        d3 = np.zeros(n, bool)
        d1[:-1] = dup[1:]
        if n > 2:
            d2[:-2] = dup[1:-1] & dup[2:]
        if n > 3:
            d3[:-3] = dup[1:-2] & dup[2:-1] & dup[3:]
            assert not (dup[1:-3] & dup[2:-2] & dup[3:-1] & dup[4:]).any(), \
                "duplicate run > 4"

        wperm = np.zeros((N, SLOTP), np.float32)
        idx = np.full((N, SLOTP), -1, np.int16)
        m1 = np.zeros((N, SLOTP), np.float16)
        m2 = np.zeros((N, SLOTP), np.float16)
        m3 = np.zeros((N, SLOTP), np.float16)
        wperm[r, slot] = ww
        m1[r, slot] = d1
        m2[r, slot] = d2
        m3[r, slot] = d3
        nd = ~dup
        idx[r[nd], slot[nd]] = c[nd].astype(np.int16)
        out.append(dict(wperm=wperm, idx=idx, m1=m1, m2=m2, m3=m3))
    return out


def _prep_weights(params, routing):
    """Shared (replicated) weight tensors, padded/cast on host."""
    def pad16(a, rows, cols, scale=1.0):
        a = np.asarray(a, np.float32) * scale
        o = np.zeros((rows, cols), np.float32)
        o[: a.shape[0], : a.shape[1]] = a
        return o.astype(np.float16)

    def padb(a, rows, scale=1.0):
        a = np.asarray(a, np.float32).reshape(-1, 1) * scale
        o = np.zeros((rows, 1), np.float32)
        o[: a.shape[0]] = a
        return o

    t = {}
    for pre, S in (("t", S_TOPO), ("f", S_TRAF)):
        # layer biases get the branch scale (they are all zero anyway)
        for i in range(4):
            nmW, nmb = f"{pre}g{i}W", f"{pre}g{i}b"
            if pre == "t" and i == 0:
                t["tg0W16"] = pad16(params[nmW], 1, FP)
            elif pre == "f" and i == 0:
                # traffic g0: X0 = dinv * (eye @ W0) = dinv * W0, pre-scaled
                t["fg0W16"] = pad16(params[nmW], N, FP, scale=S)
            else:
                t[f"{pre}g{i}W16"] = pad16(params[nmW], FP, FP)
            t[f"{pre}g{i}b"] = padb(params[nmb], FP, scale=S)
        for i in range(3):
            t[f"{pre}l{i}W16"] = pad16(params[f"{pre}l{i}W"], FP, FP)
            t[f"{pre}l{i}b"] = padb(params[f"{pre}l{i}b"], FP, scale=S)

    # head: j0 rows chunked so K-chunks align with the padded cat layout;
    # j0 cols / j1 rows are sharded per core (t["j0Ws_k"] etc. are lists).
    j0 = np.asarray(params["j0W"], np.float32)     # [600, 1024]
    j0s = np.zeros((768, 1024), np.float32)
    j0s[0:200] = j0[0:200]        # topo rows at cat[0:256]
    j0s[256:456] = j0[200:400]    # traffic rows at cat[256:512]
    j0s[512:712] = j0[400:600]    # routing rows at cat[512:768]
    j0b = np.asarray(params["j0b"], np.float32).reshape(1024)
    j1 = np.asarray(params["j1W"], np.float32)     # [1024, 512]
    t["j0Ws_k"] = [np.ascontiguousarray(j0s[:, k * 128:(k + 1) * 128])
                   for k in range(NCORES)]
    t["j0b_k"] = [np.ascontiguousarray(j0b[k * 128:(k + 1) * 128].reshape(1, 128))
                  for k in range(NCORES)]
    t["j1W_k"] = [np.ascontiguousarray(j1[k * 128:(k + 1) * 128, :])
                  for k in range(NCORES)]
    t["j1b"] = np.asarray(params["j1b"], np.float32).reshape(1, 512)
    t["r0W"] = np.asarray(params["r0W"], np.float32)          # [100, 50]
    t["r0b"] = padb(params["r0b"], 50)
    t["r1W"] = np.asarray(params["r1W"], np.float32)
    t["r1b"] = padb(params["r1b"], 50)
    r2 = np.zeros((50, FP), np.float32)
    r2[:, :200] = np.asarray(params["r2W"], np.float32)
    t["r2W"] = r2
    t["r2b"] = padb(params["r2b"], FP)
    t["routing"] = np.asarray(routing, np.float32).reshape(100, 1)
    return t


# --------------------------------------------------------------------------
# Device kernel
# --------------------------------------------------------------------------

def _build_kernel():
    nc = bacc.Bacc("TRN2", target_bir_lowering=False, debug=False,
                   num_devices=NCORES)
    RG = [list(range(NCORES))]

    dram_in = {}

    def din(name, shape, dtype):
        dram_in[name] = nc.dram_tensor(name, shape, dtype, kind="ExternalInput")
        return dram_in[name]

    for b in ("t", "f"):
        din(f"{b}_wperm", [N, SLOTP], F32)
        din(f"{b}_idx", [N, SLOTP], I16)
        din(f"{b}_m1", [N, SLOTP], F16)
        din(f"{b}_m2", [N, SLOTP], F16)
        din(f"{b}_m3", [N, SLOTP], F16)
    din("tg0W16", [1, FP], F16)
    din("fg0W16", [N, FP], F16)
    for b in ("t", "f"):
        for i in (1, 2, 3):
            din(f"{b}g{i}W16", [FP, FP], F16)
        for i in range(3):
            din(f"{b}l{i}W16", [FP, FP], F16)
        for i in range(4):
            din(f"{b}g{i}b", [FP, 1], F32)
        for i in range(3):
            din(f"{b}l{i}b", [FP, 1], F32)
    din("j0Ws_k", [768, 128], F32)
    din("j0b_k", [1, 128], F32)
    din("j1W_k", [128, 512], F32)
    din("j1b", [1, 512], F32)
    din("r0W", [100, 50], F32)
    din("r0b", [50, 1], F32)
    din("r1W", [50, 50], F32)
    din("r1b", [50, 1], F32)
    din("r2W", [50, FP], F32)
    din("r2b", [FP, 1], F32)
    din("routing", [100, 1], F32)
    out_dram = nc.dram_tensor("out", [1, 512], F32, kind="ExternalOutput")

    with tile.TileContext(nc) as tc:
        with (
            tc.tile_pool(name="persist", bufs=1) as pp,
            tc.tile_pool(name="build", bufs=1) as bp,
            tc.tile_pool(name="work", bufs=1) as wp,
            tc.tile_pool(name="xpool", bufs=1) as xp,
            tc.tile_pool(name="hpool", bufs=2) as hp,
            tc.tile_pool(name="psA", bufs=2, space="PSUM") as psA,
            tc.tile_pool(name="psB", bufs=2, space="PSUM") as psB,
            tc.tile_pool(name="psS", bufs=2, space="PSUM") as psS,
            tc.tile_pool(name="dram", bufs=1, space="DRAM") as dp,
        ):
            ones16 = pp.tile([128, 1], F16, tag="ones16")
            nc.vector.memset(ones16[:], 1.0)
            ones32r = pp.tile([1, 128], F32, tag="ones32r")
            nc.vector.memset(ones32r[:], 1.0)
            onescol = pp.tile([128, 1], F32, tag="onescol")
            nc.vector.memset(onescol[:], 1.0)

            # ---------- load shared weights ----------
            W16 = {}
            for b in ("t", "f"):
                for nm in [f"{b}g{i}W16" for i in (1, 2, 3)] + \
                          [f"{b}l{i}W16" for i in range(3)]:
                    tl = pp.tile([128, 2, FP], F16, tag=nm)
                    nc.sync.dma_start(
                        tl[:], dram_in[nm].ap().rearrange("(kh p) n -> p kh n", p=128))
                    W16[nm] = tl
            tg0W16 = pp.tile([1, FP], F16, tag="tg0W16")
            nc.sync.dma_start(tg0W16[:], dram_in["tg0W16"][:, :])
            B = {}
            for b in ("t", "f"):
                for nm in [f"{b}g{i}b" for i in range(4)] + \
                          [f"{b}l{i}b" for i in range(3)]:
                    tl = pp.tile([128, 2], F32, tag=nm)
                    nc.sync.dma_start(
                        tl[:], dram_in[nm].ap().rearrange("(h p) one -> p (h one)", p=128))
                    B[nm] = tl

            j0Ws = pp.tile([128, 6, 128], F32, tag="j0Ws")
            nc.sync.dma_start(
                j0Ws[:], dram_in["j0Ws_k"].ap().rearrange("(q p) n -> p q n", p=128))
            j1Wk = pp.tile([128, 512], F32, tag="j1Wk")
            nc.sync.dma_start(j1Wk[:], dram_in["j1W_k"][:, :])
            j0bk = pp.tile([1, 128], F32, tag="j0bk")
            nc.sync.dma_start(j0bk[:], dram_in["j0b_k"][:, :])
            j1b = pp.tile([1, 512], F32, tag="j1b")
            nc.sync.dma_start(j1b[:], dram_in["j1b"][:, :])
            r0W = pp.tile([100, 50], F32, tag="r0W")
            nc.sync.dma_start(r0W[:], dram_in["r0W"][:, :])
            r1W = pp.tile([50, 50], F32, tag="r1W")
            nc.sync.dma_start(r1W[:], dram_in["r1W"][:, :])
            r2W = pp.tile([50, FP], F32, tag="r2W")
            nc.sync.dma_start(r2W[:], dram_in["r2W"][:, :])
            r0b = pp.tile([50, 1], F32, tag="r0b")
            nc.sync.dma_start(r0b[:], dram_in["r0b"][:, :])
            r1b = pp.tile([50, 1], F32, tag="r1b")
            nc.sync.dma_start(r1b[:], dram_in["r1b"][:, :])
            r2b = pp.tile([128, 2], F32, tag="r2b")
            nc.sync.dma_start(
                r2b[:], dram_in["r2b"].ap().rearrange("(h p) one -> p (h one)", p=128))
            routing_sb = pp.tile([100, 1], F32, tag="routing")
            nc.sync.dma_start(routing_sb[:], dram_in["routing"][:, :])

            # ---------- per-branch state ----------
            state = {}

            def build_branch(b):
                """Dense A tiles + dinv for branch b (build tags shared
                between branches -> builds serialize, convs still overlap)."""
                wperm = bp.tile([128, NT, SLOTP], F32, tag="wperm")
                nc.sync.dma_start(
                    wperm[:],
                    dram_in[f"{b}_wperm"].ap().rearrange("(t p) s -> p t s", p=128))
                idx = bp.tile([128, NT, SLOTP], I16, tag="idx")
                nc.sync.dma_start(
                    idx[:],
                    dram_in[f"{b}_idx"].ap().rearrange("(t p) s -> p t s", p=128))
                masks = []
                for mi in (1, 2, 3):
                    m = bp.tile([128, NT, SLOTP], F16, tag=f"m{mi}")
                    nc.sync.dma_start(
                        m[:],
                        dram_in[f"{b}_m{mi}"].ap().rearrange("(t p) s -> p t s", p=128))
                    masks.append(m)

                # fold duplicates: weff = w + m1*sh1(w) + m2*sh2(w) + m3*sh3(w)
                w16 = bp.tile([128, NT, SLOTP], F16, tag="w16")
                nc.vector.tensor_copy(out=w16[:], in_=wperm[:])
                weff = bp.tile([128, NT, SLOTP], F16, tag="weff")
                nc.vector.tensor_copy(out=weff[:], in_=w16[:])
                tmp = bp.tile([128, NT, SLOTP], F16, tag="wtmp")
                for sh, m in ((1, masks[0]), (2, masks[1]), (3, masks[2])):
                    nc.vector.tensor_tensor(
                        out=tmp[:, :, 0:32], in0=w16[:, :, sh:32 + sh],
                        in1=m[:, :, 0:32], op=ALU.mult)
                    nc.vector.tensor_tensor(
                        out=weff[:, :, 0:32], in0=weff[:, :, 0:32],
                        in1=tmp[:, :, 0:32], op=ALU.add)

                A = []
                for t in range(NT):
                    At = pp.tile([128, NB], F16, tag=f"A_{b}_{t}")
                    nc.gpsimd.local_scatter(
                        out_ap=At[:], data_ap=weff[:, t, :], idxs_ap=idx[:, t, :],
                        channels=128, num_elems=NB, num_idxs=SLOTP)
                    A.append(At)

                # deg = ones^T @ A   -> [1, NB] f32
                deg_ps = psS.tile([1, NB], F32, tag="small")
                for t in range(NT):
                    nc.tensor.matmul(deg_ps[:], lhsT=ones16[:], rhs=A[t][:],
                                     start=(t == 0), stop=(t == NT - 1))
                sq = wp.tile([1, NB], F32, tag="sq")
                nc.scalar.activation(sq[:], deg_ps[:], ACTF.Sqrt)
                dinv_loc = pp.tile([1, NB], F32, tag=f"dinvloc_{b}")
                nc.vector.reciprocal(dinv_loc[:], sq[:])

                # dinv_rep = ones (x) dinv_loc  -> [128, NB]
                rep_ps = psA.tile([128, NB], F32, tag=f"msgps_{b}")
                nc.tensor.matmul(rep_ps[:], lhsT=ones32r[:],
                                 rhs=dinv_loc[:], start=True, stop=True)
                dinv_rep16 = pp.tile([128, NB], F16, tag=f"dinvrep_{b}")
                nc.vector.tensor_copy(out=dinv_rep16[:], in_=rep_ps[:])

                # AllGather dinv
                d_loc = dp.tile([1, NB], F32, tag=f"dloc_{b}")
                d_all = dp.tile([NCORES, NB], F32, tag=f"dall_{b}")
                nc.sync.dma_start(d_loc[:], dinv_loc[:])
                nc.gpsimd.collective_compute(
                    "AllGather", ALU.bypass, ins=[d_loc[:].opt()],
                    outs=[d_all[:].opt()], replica_groups=RG)
                dinv_pcol = pp.tile([128, NT], F32, tag=f"dinvpcol_{b}")
                nc.sync.dma_start(
                    dinv_pcol[:], d_all[:].rearrange("k (x p) -> p (k x)", p=128, x=4))
                state[b] = dict(A=A, dinv_loc=dinv_loc, dinv_rep16=dinv_rep16,
                                dinv_pcol=dinv_pcol, wperm=wperm)

            def conv_msg(b, X, bias):
                """out^T[fh] = relu(dinv_rep * (X^T @ A) + bias) -> 2x[128,NB] f16."""
                st = state[b]
                hT = []
                for fh in range(2):
                    ps = psA.tile([128, NB], F32, tag=f"msgps_{b}")
                    for t in range(NT):
                        nc.tensor.matmul(
                            ps[:], lhsT=X[t][:, fh * 128:(fh + 1) * 128],
                            rhs=st["A"][t][:], start=(t == 0), stop=(t == NT - 1))
                    tmp = wp.tile([128, NB], F32, tag=f"cvt_{b}")
                    nc.vector.tensor_tensor(out=tmp[:], in0=ps[:],
                                            in1=st["dinv_rep16"][:], op=ALU.mult)
                    h = wp.tile([128, NB], F16, tag=f"hT_{b}_{fh}")
                    nc.vector.tensor_scalar(
                        out=h[:], in0=tmp[:], scalar1=bias[:, fh:fh + 1],
                        scalar2=0.0, op0=ALU.add, op1=ALU.max)
                    hT.append(h)
                return hT

            def allgather_h(b, hT, round_id):
                """Scale by dinv_loc, AllGather -> hTfull 2x[128, 8, NB] f16."""
                st = state[b]
                bh_loc = dp.tile([2, 128, NB], F16, tag=f"agl_{b}_{round_id}")
                bh_all = dp.tile([NCORES, 2, 128, NB], F16,
                                 tag=f"aga_{b}_{round_id}")
                for fh in range(2):
                    hs = wp.tile([128, NB], F16, tag=f"hsc_{b}")
                    nc.vector.tensor_tensor(out=hs[:], in0=hT[fh][:],
                                            in1=st["dinv_rep16"][:], op=ALU.mult)
                    nc.sync.dma_start(bh_loc[fh, :, :], hs[:])
                nc.gpsimd.collective_compute(
                    "AllGather", ALU.bypass, ins=[bh_loc[:].opt()],
                    outs=[bh_all[:].opt()], replica_groups=RG)
                return bh_all

            def wmatmul(b, bh_all, Wnm):
                """X[t] = hT-slices @ W, streaming AG output per rank chunk."""
                W = W16[Wnm]
                X = []
                for k in range(NCORES):
                    hTk = []
                    for fh in range(2):
                        hc = hp.tile([128, NB], F16, tag=f"hTk_{b}_{fh}")
                        nc.sync.dma_start(hc[:], bh_all[k, fh, :, :])
                        hTk.append(hc)
                    for j in range(4):
                        ps = psB.tile([128, FP], F32, tag="xps")
                        for kh in range(2):
                            nc.tensor.matmul(
                                ps[:], lhsT=hTk[kh][:, j * 128:(j + 1) * 128],
                                rhs=W[:, kh, :], start=(kh == 0), stop=(kh == 1))
                        Xt = xp.tile([128, FP], F16, tag=f"X_{b}_{k}_{j}")
                        nc.vector.tensor_copy(out=Xt[:], in_=ps[:])
                        X.append(Xt)
                return X

            def lin_local(b, hT, Wnm, bias, relu, out_dtype=F16):
                """yT[mh] = act(W^T @ hT + b), sharded [128, NB]."""
                W = W16[Wnm]
                out = []
                for mh in range(2):
                    ps = psA.tile([128, NB], F32, tag=f"msgps_{b}")
                    for kh in range(2):
                        nc.tensor.matmul(
                            ps[:], lhsT=W[:, kh, mh * 128:(mh + 1) * 128],
                            rhs=hT[kh][:], start=(kh == 0), stop=(kh == 1))
                    y = wp.tile([128, NB], out_dtype, tag=f"lin_{b}_{mh}")
                    if relu:
                        nc.vector.tensor_scalar(
                            out=y[:], in0=ps[:], scalar1=bias[:, mh:mh + 1],
                            scalar2=0.0, op0=ALU.add, op1=ALU.max)
                    else:
                        nc.vector.tensor_scalar_add(
                            out=y[:], in0=ps[:], scalar1=bias[:, mh:mh + 1])
                    out.append(y)
                return out

            # ================= branch computation (stage-interleaved) ======
            def stage_build(b):
                build_branch(b)
                st = state[b]
                if b == "t":
                    # topo feature: rowsum of raw w (incl +1 self loop)
                    rowsum = wp.tile([128, NT], F32, tag="rowsum")
                    nc.vector.reduce_sum(out=rowsum[:], in_=st["wperm"][:], axis=AX)
                    rs_loc = dp.tile([N, 1], F32, tag="rsloc")
                    rs_sum = dp.tile([N, 1], F32, tag="rssum")
                    nc.sync.dma_start(
                        rs_loc[:].rearrange("(t p) one -> p (t one)", p=128),
                        rowsum[:])
                    nc.gpsimd.collective_compute(
                        "AllReduce", ALU.add, ins=[rs_loc[:].opt()],
                        outs=[rs_sum[:].opt()], replica_groups=RG)
                    st["rs_sum"] = rs_sum

            def stage_g0(b):
                st = state[b]
                if b == "t":
                    rs_glob = wp.tile([128, NT], F32, tag="rsglob")
                    nc.sync.dma_start(
                        rs_glob[:],
                        st["rs_sum"][:].rearrange("(t p) one -> p (t one)", p=128))
                    # tot = sum(rs_glob) ; stot = S_TOPO / (tot - 4096)
                    rsr = wp.tile([128, 1], F32, tag="rsr")
                    nc.vector.reduce_sum(out=rsr[:], in_=rs_glob[:], axis=AX)
                    tot_ps = psS.tile([1, 1], F32, tag="small")
                    nc.tensor.matmul(tot_ps[:], lhsT=rsr[:],
                                     rhs=onescol[:], start=True, stop=True)
                    stot = wp.tile([1, 1], F32, tag="stot")
                    nc.vector.tensor_scalar_add(out=stot[:], in0=tot_ps[:],
                                                scalar1=-4096.0)
                    nc.vector.reciprocal(stot[:], stot[:])
                    nc.vector.tensor_scalar_mul(out=stot[:], in0=stot[:],
                                                scalar1=S_TOPO)
                    # v = dinv_pcol * (rs_glob - 1)
                    v = wp.tile([128, NT], F32, tag="vfeat")
                    nc.vector.tensor_scalar_add(out=v[:], in0=rs_glob[:],
                                                scalar1=-1.0)
                    nc.vector.tensor_tensor(out=v[:], in0=v[:],
                                            in1=st["dinv_pcol"][:], op=ALU.mult)
                    v16 = wp.tile([128, NT], F16, tag="v16")
                    nc.vector.tensor_copy(out=v16[:], in_=v[:])
                    # u = v^T @ A -> [1, NB];  uu = u * dinv_loc * stot
                    u_ps = psS.tile([1, NB], F32, tag="small")
                    for t in range(NT):
                        nc.tensor.matmul(u_ps[:], lhsT=v16[:, t:t + 1],
                                         rhs=st["A"][t][:],
                                         start=(t == 0), stop=(t == NT - 1))
                    uu = wp.tile([1, NB], F32, tag="uu")
                    nc.vector.tensor_tensor(out=uu[:], in0=u_ps[:],
                                            in1=st["dinv_loc"][:], op=ALU.mult)
                    nc.vector.tensor_scalar_mul(out=uu[:], in0=uu[:],
                                                scalar1=stot[0:1, 0:1])
                    uu16 = wp.tile([1, NB], F16, tag="uu16")
                    nc.vector.tensor_copy(out=uu16[:], in_=uu[:])
                    # h0T[fh] = relu(tg0W[fh] (x) uu + b)
                    hT = []
                    for fh in range(2):
                        ps = psA.tile([128, NB], F32, tag=f"msgps_{b}")
                        nc.tensor.matmul(
                            ps[:], lhsT=tg0W16[:, fh * 128:(fh + 1) * 128],
                            rhs=uu16[:], start=True, stop=True)
                        h = wp.tile([128, NB], F16, tag=f"hT_{b}_{fh}")
                        nc.vector.tensor_scalar(
                            out=h[:], in0=ps[:],
                            scalar1=B["tg0b"][:, fh:fh + 1], scalar2=0.0,
                            op0=ALU.add, op1=ALU.max)
                        hT.append(h)
                else:
                    # traffic g0: X0 = dinv * W0 (W0 pre-scaled by S_TRAF)
                    X = []
                    for t in range(NT):
                        Xt = xp.tile([128, FP], F16, tag=f"X_{b}_{t}")
                        nc.sync.dma_start(
                            Xt[:],
                            dram_in["fg0W16"].ap().rearrange(
                                "(t p) n -> p t n", p=128)[:, t, :])
                        nc.vector.tensor_scalar_mul(
                            out=Xt[:], in0=Xt[:],
                            scalar1=st["dinv_pcol"][:, t:t + 1])
                        X.append(Xt)
                    hT = conv_msg(b, X, B["fg0b"])
                st["hT"] = hT

            def stage_conv(b, g, lin_before=None, lin_after=()):
                st = state[b]
                hT = st["hT"]
                if lin_before:
                    hT = lin_local(b, hT, f"{b}{lin_before}W16",
                                   B[f"{b}{lin_before}b"], True)
                hTf = allgather_h(b, hT, g)
                X = wmatmul(b, hTf, f"{b}{g}W16")
                hT = conv_msg(b, X, B[f"{b}{g}b"])
                st["hT"] = hT

            def stage_tail(b):
                st = state[b]
                hT = lin_local(b, st["hT"], f"{b}l1W16", B[f"{b}l1b"], True)
                yT = lin_local(b, hT, f"{b}l2W16", B[f"{b}l2b"], False,
                               out_dtype=F32)
                S = S_TOPO if b == "t" else S_TRAF
                mp = []
                for fh in range(2):
                    m = wp.tile([128, 1], F32, tag=f"mean_{b}_{fh}")
                    nc.vector.reduce_sum(out=m[:], in_=yT[fh][:], axis=AX)
                    nc.vector.tensor_scalar_mul(out=m[:], in0=m[:],
                                                scalar1=1.0 / (4096.0 * S))
                    mp.append(m)
                return mp

            stage_build("t")
            stage_build("f")
            stage_g0("t")
            stage_g0("f")
            for b in ("t", "f"):
                stage_conv(b, "g1")
            for b in ("t", "f"):
                stage_conv(b, "g2", lin_before="l0")
            for b in ("t", "f"):
                stage_conv(b, "g3")
            mp_t = stage_tail("t")
            mp_f = stage_tail("f")

            # ---------------- final AllReduce of branch means ----------------
            fin_loc = dp.tile([512, 1], F32, tag="finloc")
            fin_sum = dp.tile([512, 1], F32, tag="finsum")
            for j, m in enumerate(mp_t + mp_f):
                nc.sync.dma_start(fin_loc[j * 128:(j + 1) * 128, :], m[:])
            nc.gpsimd.collective_compute(
                "AllReduce", ALU.add, ins=[fin_loc[:].opt()],
                outs=[fin_sum[:].opt()], replica_groups=RG)
            fin_sb = wp.tile([128, 4], F32, tag="finsb")
            nc.sync.dma_start(
                fin_sb[:], fin_sum[:].rearrange("(q p) one -> p (q one)", p=128))

            # ---------------- routing MLP (f32, redundant per core) ----------
            ps = psS.tile([50, 1], F32, tag="small")
            nc.tensor.matmul(ps[:], lhsT=r0W[:], rhs=routing_sb[:],
                             start=True, stop=True)
            y0 = wp.tile([50, 1], F32, tag="y0")
            nc.scalar.activation(y0[:], ps[:], ACTF.Relu, bias=r0b[:])
            ps = psS.tile([50, 1], F32, tag="small")
            nc.tensor.matmul(ps[:], lhsT=r1W[:], rhs=y0[:], start=True, stop=True)
            y1r = wp.tile([50, 1], F32, tag="y1r")
            nc.scalar.activation(y1r[:], ps[:], ACTF.Relu, bias=r1b[:])
            rvec = wp.tile([128, 2], F32, tag="rvec")
            for mh in range(2):
                ps = psS.tile([128, 1], F32, tag="small")
                nc.tensor.matmul(ps[:], lhsT=r2W[:, mh * 128:(mh + 1) * 128],
                                 rhs=y1r[:], start=True, stop=True)
                nc.vector.tensor_scalar_add(out=rvec[:, mh:mh + 1], in0=ps[:],
                                            scalar1=r2b[:, mh:mh + 1])

            # cat layout [128, 6]: fin(4 chunks) + rvec(2 chunks)
            cat = wp.tile([128, 6], F32, tag="cat")
            nc.vector.tensor_copy(out=cat[:, 0:4], in_=fin_sb[:])
            nc.vector.tensor_copy(out=cat[:, 4:6], in_=rvec[:])

            # head j0 (col-sharded): y1_blk = relu(cat @ j0Ws_k + j0b_k) [1,128]
            ps = psS.tile([1, 128], F32, tag="small")
            for q in range(6):
                nc.tensor.matmul(
                    ps[:], lhsT=cat[:, q:q + 1],
                    rhs=j0Ws[:, q, :],
                    start=(q == 0), stop=(q == 5))
            y1b = wp.tile([1, 128], F32, tag="y1b")
            nc.vector.tensor_tensor(out=y1b[:], in0=ps[:], in1=j0bk[:], op=ALU.add)
            nc.vector.tensor_scalar_max(out=y1b[:], in0=y1b[:], scalar1=0.0)
            # transpose y1_blk via DRAM bounce -> [128, 1]
            y1d = dp.tile([128, 1], F32, tag="y1d")
            nc.sync.dma_start(y1d[:].rearrange("a one -> one a"), y1b[:])
            y1col = wp.tile([128, 1], F32, tag="y1col")
            nc.sync.dma_start(y1col[:], y1d[:])
            # j1 partial (row-sharded): part = y1_blk @ j1W_k  [1, 512]
            ps = psS.tile([1, 512], F32, tag="small")
            nc.tensor.matmul(ps[:], lhsT=y1col[:],
                             rhs=j1Wk[:], start=True, stop=True)
            y2p = wp.tile([1, 512], F32, tag="y2p")
            nc.vector.tensor_copy(out=y2p[:], in_=ps[:])
            h_loc = dp.tile([1, 512], F32, tag="hloc")
            h_sum = dp.tile([1, 512], F32, tag="hsum")
            nc.sync.dma_start(h_loc[:], y2p[:])
            nc.gpsimd.collective_compute(
                "AllReduce", ALU.add, ins=[h_loc[:].opt()],
                outs=[h_sum[:].opt()], replica_groups=RG)
            yf = wp.tile([1, 512], F32, tag="yf")
            nc.sync.dma_start(yf[:], h_sum[:])
            nc.vector.tensor_tensor(out=yf[:], in0=yf[:], in1=j1b[:], op=ALU.add)
            nc.sync.dma_start(out_dram[:, :], yf[:])

    nc.compile()
    return nc


_NC = None


def _get_nc():
    global _NC
    if _NC is None:
        _NC = _build_kernel()
    return _NC


def _make_in_maps(topo_edge_index, topo_edge_weight, traffic_edge_index,
                  traffic_edge_weight, routing, params):
    prep_t = _prep_branch(topo_edge_index, topo_edge_weight)
    prep_f = _prep_branch(traffic_edge_index, traffic_edge_weight)
    shared = _prep_weights(params, routing)
    in_maps = []
    for k in range(NCORES):
        m = {kk: vv for kk, vv in shared.items()
             if kk not in ("j0Ws_k", "j0b_k", "j1W_k")}
        m["j0Ws_k"] = shared["j0Ws_k"][k]
        m["j0b_k"] = shared["j0b_k"][k]
        m["j1W_k"] = shared["j1W_k"][k]
        for b, prep in (("t", prep_t), ("f", prep_f)):
            m[f"{b}_wperm"] = prep[k]["wperm"]
            m[f"{b}_idx"] = prep[k]["idx"]
            m[f"{b}_m1"] = prep[k]["m1"]
            m[f"{b}_m2"] = prep[k]["m2"]
            m[f"{b}_m3"] = prep[k]["m3"]
        in_maps.append(m)
    return in_maps


def run(inputs, trace=False, trace_kwargs=None):
    nc = _get_nc()
    in_maps = _make_in_maps(**inputs)
    res = run_bass_kernel_spmd(
        nc, in_maps, core_ids=list(range(NCORES)), trace=trace,
        trace_kwargs=trace_kwargs or {})
    out = res.results[0]["out"].reshape(512).astype(np.float32)
    return out, res


def kernel(topo_edge_index, topo_edge_weight, traffic_edge_index,
           traffic_edge_weight, routing, params):
    out, _ = run(dict(topo_edge_index=topo_edge_index,
                      topo_edge_weight=topo_edge_weight,
                      traffic_edge_index=traffic_edge_index,
                      traffic_edge_weight=traffic_edge_weight,
                      routing=routing, params=params))
    return out


# revision 12
# speedup vs baseline: 1.0315x; 1.0315x over previous
"""Trainium2 Bass kernel for nn_GCNNetwork (2-branch GCN + MLP head), 8 NeuronCores.

Strategy
--------
Each of the 8 cores owns a block of 512 destination nodes (dst block k =
nodes [512k, 512k+512)).  On the host we only do *index* preprocessing and a
pure permutation of the edge-weight values:

  - append self-loop edges (w=1), assign each edge to the core owning its
    destination, sort each core's edges by (src row, dst col), and lay the
    weight values out in a padded per-source-row slot array  w_perm[4096, 36]
    together with an int16 slot->dst-column index array (-1 = unused slot and
    duplicate-continuation slots) plus duplicate-fold masks.

On device, each core:
  - folds duplicate (src,dst) edges with shifted adds (all arithmetic on
    device), then builds its dense fp16 adjacency block A[4096, 512] with 32
    `local_scatter` ops (per-partition indexed scatter, auto-zeroing),
  - computes degrees as ones^T @ A (PSUM f32), dinv = 1/sqrt(deg),
    AllGathers dinv,
  - runs every GCN conv as dense matmuls:  out^T = X^T @ A  with
    X = (dinv * h) @ W, normalization folded into cheap row/col scalings,
  - AllGathers the dinv-scaled 200-wide activations between convs; linear
    layers and the node-mean stay sharded; one final AllReduce combines the
    branch means, and the small MLP head runs redundantly on every core in
    float32r.

All-zero biases + relu make each branch positively homogeneous, so branch
inputs are scaled by 2^20 (topo) / 2^14 (traffic) to keep fp16 activations
out of the subnormal range; the scale is removed exactly (power of two) when
the node-mean is taken.
"""

import os
import sys

import numpy as np

for _p in ("/opt/trn_rl_repo", "/root/.axon_site/_ro/trn_rl_repo"):
    if os.path.isdir(_p) and _p not in sys.path:
        sys.path.append(_p)

import concourse.bacc as bacc
import concourse.bass as bass
import concourse.mybir as mybir
import concourse.tile as tile
from concourse.bass_utils import run_bass_kernel_spmd

N = 4096          # nodes
NB = 512          # dst nodes per core
NCORES = 8
NT = 32           # src tiles of 128
F = 200           # true feature width
FP = 256          # padded feature width
SLOTS = 32        # max edges (src row -> this core's dst block), asserted
SLOTP = 36        # slot array width (shift padding, even)
S_TOPO = float(2 ** 20)
S_TRAF = float(2 ** 14)

F16 = mybir.dt.float16
F32 = mybir.dt.float32
F32R = mybir.dt.float32r
I16 = mybir.dt.int16
AX = mybir.AxisListType.X
ALU = mybir.AluOpType
ACTF = mybir.ActivationFunctionType


# --------------------------------------------------------------------------
# Host-side: index preprocessing + pure value permutation (no arithmetic).
# --------------------------------------------------------------------------

def _prep_branch(edge_index, edge_weight):
    """Shard edges by dst block; per core build padded slot layout.

    Returns list (per core) of dicts: wperm f32 [N, SLOTP], idx int16
    [N, SLOTP], m1/m2/m3 fp16 [N, SLOTP].
    """
    row = np.asarray(edge_index[0]).astype(np.int64)
    col = np.asarray(edge_index[1]).astype(np.int64)
    w = np.asarray(edge_weight).astype(np.float32)
    ar = np.arange(N, dtype=np.int64)
    row = np.concatenate([row, ar])
    col = np.concatenate([col, ar])
    w = np.concatenate([w, np.ones(N, np.float32)])

    out = []
    for k in range(NCORES):
        sel = (col >> 9) == k
        r = row[sel]
        c = col[sel] - NB * k
        ww = w[sel]
        order = np.lexsort((c, r))
        r, c, ww = r[order], c[order], ww[order]
        n = len(r)

        new_row = np.empty(n, bool)
        new_row[0] = True
        new_row[1:] = r[1:] != r[:-1]
        first_of_row = np.maximum.accumulate(np.where(new_row, np.arange(n), 0))
        slot = np.arange(n) - first_of_row
        assert slot.max() < SLOTS, f"slot overflow: {slot.max()}"

        dup = np.zeros(n, bool)
        dup[1:] = (r[1:] == r[:-1]) & (c[1:] == c[:-1])
        d1 = np.zeros(n, bool)
        d2 = np.zeros(n, bool)
        d3 = np.zeros(n, bool)
        d1[:-1] = dup[1:]
        if n > 2:
            d2[:-2] = dup[1:-1] & dup[2:]
        if n > 3:
            d3[:-3] = dup[1:-2] & dup[2:-1] & dup[3:]
            assert not (dup[1:-3] & dup[2:-2] & dup[3:-1] & dup[4:]).any(), \
                "duplicate run > 4"

        wperm = np.zeros((N, SLOTP), np.float32)
        idx = np.full((N, SLOTP), -1, np.int16)
        m1 = np.zeros((N, SLOTP), np.float16)
        m2 = np.zeros((N, SLOTP), np.float16)
        m3 = np.zeros((N, SLOTP), np.float16)
        wperm[r, slot] = ww
        m1[r, slot] = d1
        m2[r, slot] = d2
        m3[r, slot] = d3
        nd = ~dup
        idx[r[nd], slot[nd]] = c[nd].astype(np.int16)
        out.append(dict(wperm=wperm, idx=idx, m1=m1, m2=m2, m3=m3))
    return out


def _prep_weights(params, routing):
    """Shared (replicated) weight tensors, padded/cast on host."""
    def pad16(a, rows, cols, scale=1.0):
        a = np.asarray(a, np.float32) * scale
        o = np.zeros((rows, cols), np.float32)
        o[: a.shape[0], : a.shape[1]] = a
        return o.astype(np.float16)

    def padb(a, rows, scale=1.0):
        a = np.asarray(a, np.float32).reshape(-1, 1) * scale
        o = np.zeros((rows, 1), np.float32)
        o[: a.shape[0]] = a
        return o

    t = {}
    wstack = []   # 12 x [FP, FP] fp16, order fixed
    bstack = []   # 14 x [FP, 1] f32
    for pre, S in (("t", S_TOPO), ("f", S_TRAF)):
        # layer biases get the branch scale (they are all zero anyway)
        for i in range(4):
            nmW, nmb = f"{pre}g{i}W", f"{pre}g{i}b"
            if pre == "t" and i == 0:
                t["tg0W16"] = pad16(params[nmW], 1, FP)
            elif pre == "f" and i == 0:
                # traffic g0: X0 = dinv * (eye @ W0) = dinv * W0, pre-scaled
                t["fg0W16"] = pad16(params[nmW], N, FP, scale=S)
            else:
                wstack.append(pad16(params[nmW], FP, FP))
            bstack.append(padb(params[nmb], FP, scale=S))
        for i in range(3):
            wstack.append(pad16(params[f"{pre}l{i}W"], FP, FP))
            bstack.append(padb(params[f"{pre}l{i}b"], FP, scale=S))
    t["Wstack"] = np.concatenate(wstack, axis=0)      # [12*FP, FP] fp16
    t["Bstack"] = np.concatenate(bstack, axis=1)      # [FP, 14] f32

    # head: j0 rows chunked so K-chunks align with the padded cat layout;
    # j0 cols / j1 rows are sharded per core (t["j0Ws_k"] etc. are lists).
    j0 = np.asarray(params["j0W"], np.float32)     # [600, 1024]
    j0s = np.zeros((768, 1024), np.float32)
    j0s[0:200] = j0[0:200]        # topo rows at cat[0:256]
    j0s[256:456] = j0[200:400]    # traffic rows at cat[256:512]
    j0s[512:712] = j0[400:600]    # routing rows at cat[512:768]
    j0b = np.asarray(params["j0b"], np.float32).reshape(1024)
    j1 = np.asarray(params["j1W"], np.float32)     # [1024, 512]
    t["j0Ws_k"] = [np.ascontiguousarray(j0s[:, k * 128:(k + 1) * 128])
                   for k in range(NCORES)]
    t["j0b_k"] = [np.ascontiguousarray(j0b[k * 128:(k + 1) * 128].reshape(1, 128))
                  for k in range(NCORES)]
    t["j1W_k"] = [np.ascontiguousarray(j1[k * 128:(k + 1) * 128, :])
                  for k in range(NCORES)]
    t["j1b"] = np.asarray(params["j1b"], np.float32).reshape(1, 512)
    t["r0W"] = np.asarray(params["r0W"], np.float32)          # [100, 50]
    t["r0b"] = padb(params["r0b"], 50)
    t["r1W"] = np.asarray(params["r1W"], np.float32)
    t["r1b"] = padb(params["r1b"], 50)
    r2 = np.zeros((50, FP), np.float32)
    r2[:, :200] = np.asarray(params["r2W"], np.float32)
    t["r2W"] = r2
    t["r2b"] = padb(params["r2b"], FP)
    t["routing"] = np.asarray(routing, np.float32).reshape(100, 1)
    return t


# --------------------------------------------------------------------------
# Device kernel
# --------------------------------------------------------------------------

def _build_kernel():
    nc = bacc.Bacc("TRN2", target_bir_lowering=False, debug=False,
                   num_devices=NCORES)
    RG = [list(range(NCORES))]

    dram_in = {}

    def din(name, shape, dtype):
        dram_in[name] = nc.dram_tensor(name, shape, dtype, kind="ExternalInput")
        return dram_in[name]

    for b in ("t", "f"):
        din(f"{b}_wperm", [N, SLOTP], F32)
        din(f"{b}_idx", [N, SLOTP], I16)
        din(f"{b}_m1", [N, SLOTP], F16)
        din(f"{b}_m2", [N, SLOTP], F16)
        din(f"{b}_m3", [N, SLOTP], F16)
    din("tg0W16", [1, FP], F16)
    din("fg0W16", [N, FP], F16)
    din("Wstack", [12 * FP, FP], F16)
    din("Bstack", [FP, 14], F32)
    din("j0Ws_k", [768, 128], F32)
    din("j0b_k", [1, 128], F32)
    din("j1W_k", [128, 512], F32)
    din("j1b", [1, 512], F32)
    din("r0W", [100, 50], F32)
    din("r0b", [50, 1], F32)
    din("r1W", [50, 50], F32)
    din("r1b", [50, 1], F32)
    din("r2W", [50, FP], F32)
    din("r2b", [FP, 1], F32)
    din("routing", [100, 1], F32)
    out_dram = nc.dram_tensor("out", [1, 512], F32, kind="ExternalOutput")

    # stacked-weight index maps (host order in _prep_weights)
    WIDX = {}
    BIDX = {}
    for bi, b in enumerate(("t", "f")):
        for j, nm in enumerate([f"{b}g1W16", f"{b}g2W16", f"{b}g3W16",
                                f"{b}l0W16", f"{b}l1W16", f"{b}l2W16"]):
            WIDX[nm] = bi * 6 + j
        for j, nm in enumerate([f"{b}g0b", f"{b}g1b", f"{b}g2b", f"{b}g3b",
                                f"{b}l0b", f"{b}l1b", f"{b}l2b"]):
            BIDX[nm] = bi * 7 + j

    with tile.TileContext(nc) as tc:
        with (
            tc.tile_pool(name="persist", bufs=1) as pp,
            tc.tile_pool(name="build", bufs=1) as bp,
            tc.tile_pool(name="work", bufs=1) as wp,
            tc.tile_pool(name="xpool", bufs=1) as xp,
            tc.tile_pool(name="hpool", bufs=2) as hp,
            tc.tile_pool(name="psA", bufs=2, space="PSUM") as psA,
            tc.tile_pool(name="psB", bufs=2, space="PSUM") as psB,
            tc.tile_pool(name="psS", bufs=2, space="PSUM") as psS,
            tc.tile_pool(name="dram", bufs=1, space="DRAM") as dp,
        ):
            ones16 = pp.tile([128, 1], F16, tag="ones16")
            nc.vector.memset(ones16[:], 1.0)
            onescol = pp.tile([128, 1], F32, tag="onescol")
            nc.vector.memset(onescol[:], 1.0)
            ones32r = pp.tile([1, 128], F32, tag="ones32r")
            nc.vector.memset(ones32r[:], 1.0)

            # warm-up collective: absorbs ncfw/collectives first-call latency
            wu_in = dp.tile([1, 16], F32, tag="wu_in")
            wu_out = dp.tile([NCORES, 16], F32, tag="wu_out")
            warm = wp.tile([1, 16], F32, tag="warm")
            nc.vector.memset(warm[:], 0.0)
            nc.scalar.dma_start(wu_in[:], warm[:])
            nc.gpsimd.collective_compute(
                "AllGather", ALU.bypass, ins=[wu_in[:].opt()],
                outs=[wu_out[:].opt()], replica_groups=RG)

            # ---------- load shared weights (batched, on ACT queue) ----------
            Wbig = pp.tile([128, 12, 2, FP], F16, tag="Wbig")
            nc.scalar.dma_start(
                Wbig[:], dram_in["Wstack"].ap().rearrange(
                    "(w kh p) n -> p w kh n", p=128, kh=2))
            Bb = pp.tile([128, 2, 14], F32, tag="Bb")
            nc.scalar.dma_start(
                Bb[:], dram_in["Bstack"].ap().rearrange("(h p) c -> p h c", p=128))

            def Wsl(nm, kh, mh=None):
                ap = Wbig[:, WIDX[nm], kh, :]
                if mh is None:
                    return ap
                return Wbig[:, WIDX[nm], kh, mh * 128:(mh + 1) * 128]

            def Bsl(nm, fh):
                return Bb[:, fh, BIDX[nm]:BIDX[nm] + 1]

            tg0W16 = pp.tile([1, FP], F16, tag="tg0W16")
            nc.scalar.dma_start(tg0W16[:], dram_in["tg0W16"][:, :])
            j0Ws = pp.tile([128, 6, 128], F32, tag="j0Ws")
            nc.scalar.dma_start(
                j0Ws[:], dram_in["j0Ws_k"].ap().rearrange("(q p) n -> p q n", p=128))
            j1Wk = pp.tile([128, 512], F32, tag="j1Wk")
            nc.scalar.dma_start(j1Wk[:], dram_in["j1W_k"][:, :])
            j0bk = pp.tile([1, 128], F32, tag="j0bk")
            nc.scalar.dma_start(j0bk[:], dram_in["j0b_k"][:, :])
            j1b = pp.tile([1, 512], F32, tag="j1b")
            nc.scalar.dma_start(j1b[:], dram_in["j1b"][:, :])
            r0W = pp.tile([100, 50], F32, tag="r0W")
            nc.scalar.dma_start(r0W[:], dram_in["r0W"][:, :])
            r1W = pp.tile([50, 50], F32, tag="r1W")
            nc.scalar.dma_start(r1W[:], dram_in["r1W"][:, :])
            r2W = pp.tile([50, FP], F32, tag="r2W")
            nc.scalar.dma_start(r2W[:], dram_in["r2W"][:, :])
            r0b = pp.tile([50, 1], F32, tag="r0b")
            nc.scalar.dma_start(r0b[:], dram_in["r0b"][:, :])
            r1b = pp.tile([50, 1], F32, tag="r1b")
            nc.scalar.dma_start(r1b[:], dram_in["r1b"][:, :])
            r2b = pp.tile([128, 2], F32, tag="r2b")
            nc.scalar.dma_start(
                r2b[:], dram_in["r2b"].ap().rearrange("(h p) one -> p (h one)", p=128))
            routing_sb = pp.tile([100, 1], F32, tag="routing")
            nc.scalar.dma_start(routing_sb[:], dram_in["routing"][:, :])

            # ------- routing MLP (independent; runs during builds) -------
            ps = psS.tile([50, 1], F32, tag="small")
            nc.tensor.matmul(ps[:], lhsT=r0W[:], rhs=routing_sb[:],
                             start=True, stop=True)
            y0 = wp.tile([50, 1], F32, tag="y0")
            nc.scalar.activation(y0[:], ps[:], ACTF.Relu, bias=r0b[:])
            ps = psS.tile([50, 1], F32, tag="small")
            nc.tensor.matmul(ps[:], lhsT=r1W[:], rhs=y0[:], start=True, stop=True)
            y1r = wp.tile([50, 1], F32, tag="y1r")
            nc.scalar.activation(y1r[:], ps[:], ACTF.Relu, bias=r1b[:])
            rvec = pp.tile([128, 2], F32, tag="rvec")
            for mh in range(2):
                ps = psS.tile([128, 1], F32, tag="small")
                nc.tensor.matmul(ps[:], lhsT=r2W[:, mh * 128:(mh + 1) * 128],
                                 rhs=y1r[:], start=True, stop=True)
                nc.vector.tensor_scalar_add(out=rvec[:, mh:mh + 1], in0=ps[:],
                                            scalar1=r2b[:, mh:mh + 1])

            # ---------- per-branch state ----------
            state = {}

            def build_branch(b):
                """Dense A tiles + dinv for branch b (build tags shared
                between branches -> builds serialize, convs still overlap)."""
                wperm = bp.tile([128, NT, SLOTP], F32, tag="wperm")
                nc.scalar.dma_start(
                    wperm[:],
                    dram_in[f"{b}_wperm"].ap().rearrange("(t p) s -> p t s", p=128))
                idx = bp.tile([128, NT, SLOTP], I16, tag="idx")
                nc.scalar.dma_start(
                    idx[:],
                    dram_in[f"{b}_idx"].ap().rearrange("(t p) s -> p t s", p=128))
                masks = []
                for mi in (1, 2, 3):
                    m = bp.tile([128, NT, SLOTP], F16, tag=f"m{mi}")
                    nc.scalar.dma_start(
                        m[:],
                        dram_in[f"{b}_m{mi}"].ap().rearrange("(t p) s -> p t s", p=128))
                    masks.append(m)

                # fold duplicates: weff = w + m1*sh1(w) + m2*sh2(w) + m3*sh3(w)
                w16 = bp.tile([128, NT, SLOTP], F16, tag="w16")
                nc.vector.tensor_copy(out=w16[:], in_=wperm[:])
                weff = bp.tile([128, NT, SLOTP], F16, tag="weff")
                nc.vector.tensor_copy(out=weff[:], in_=w16[:])
                tmp = bp.tile([128, NT, SLOTP], F16, tag="wtmp")
                for sh, m in ((1, masks[0]), (2, masks[1]), (3, masks[2])):
                    nc.vector.tensor_tensor(
                        out=tmp[:, :, 0:32], in0=w16[:, :, sh:32 + sh],
                        in1=m[:, :, 0:32], op=ALU.mult)
                    nc.vector.tensor_tensor(
                        out=weff[:, :, 0:32], in0=weff[:, :, 0:32],
                        in1=tmp[:, :, 0:32], op=ALU.add)

                A = []
                for t in range(NT):
                    At = pp.tile([128, NB], F16, tag=f"A_{b}_{t}")
                    nc.gpsimd.local_scatter(
                        out_ap=At[:], data_ap=weff[:, t, :], idxs_ap=idx[:, t, :],
                        channels=128, num_elems=NB, num_idxs=SLOTP)
                    A.append(At)

                # deg = ones^T @ A   -> [1, NB] f32
                deg_ps = psS.tile([1, NB], F32, tag="small")
                for t in range(NT):
                    nc.tensor.matmul(deg_ps[:], lhsT=ones16[:], rhs=A[t][:],
                                     start=(t == 0), stop=(t == NT - 1))
                sq = wp.tile([1, NB], F32, tag="sq")
                nc.scalar.activation(sq[:], deg_ps[:], ACTF.Sqrt)
                dinv_loc = pp.tile([1, NB], F32, tag=f"dinvloc_{b}")
                nc.vector.reciprocal(dinv_loc[:], sq[:])

                # dinv_rep = ones (x) dinv_loc  -> [128, NB]
                rep_ps = psA.tile([128, NB], F32, tag=f"msgps_{b}")
                nc.tensor.matmul(rep_ps[:], lhsT=ones32r[:],
                                 rhs=dinv_loc[:], start=True, stop=True)
                dinv_rep16 = pp.tile([128, NB], F16, tag=f"dinvrep_{b}")
                nc.vector.tensor_copy(out=dinv_rep16[:], in_=rep_ps[:])

                # AllGather dinv
                d_loc = dp.tile([1, NB], F32, tag=f"dloc_{b}")
                d_all = dp.tile([NCORES, NB], F32, tag=f"dall_{b}")
                nc.sync.dma_start(d_loc[:], dinv_loc[:])
                nc.gpsimd.collective_compute(
                    "AllGather", ALU.bypass, ins=[d_loc[:].opt()],
                    outs=[d_all[:].opt()], replica_groups=RG)
                dinv_pcol = pp.tile([128, NT], F32, tag=f"dinvpcol_{b}")
                nc.scalar.dma_start(
                    dinv_pcol[:], d_all[:].rearrange("k (x p) -> p (k x)", p=128, x=4))
                state[b] = dict(A=A, dinv_loc=dinv_loc, dinv_rep16=dinv_rep16,
                                dinv_pcol=dinv_pcol, wperm=wperm)

            def conv_msg(b, X, bias_nm):
                """out^T[fh] = relu(dinv_rep * (X^T @ A) + bias) -> 2x[128,NB] f16."""
                st = state[b]
                hT = []
                for fh in range(2):
                    ps = psA.tile([128, NB], F32, tag=f"msgps_{b}")
                    for t in range(NT):
                        nc.tensor.matmul(
                            ps[:], lhsT=X[:, t, fh * 128:(fh + 1) * 128],
                            rhs=st["A"][t][:], start=(t == 0), stop=(t == NT - 1))
                    tmp = wp.tile([128, NB], F32, tag=f"cvt_{b}")
                    nc.vector.tensor_tensor(out=tmp[:], in0=ps[:],
                                            in1=st["dinv_rep16"][:], op=ALU.mult)
                    h = wp.tile([128, NB], F16, tag=f"hT_{b}_{fh}")
                    nc.vector.tensor_scalar(
                        out=h[:], in0=tmp[:], scalar1=Bsl(bias_nm, fh),
                        scalar2=0.0, op0=ALU.add, op1=ALU.max)
                    hT.append(h)
                return hT

            def allgather_h(b, hT, round_id):
                """Scale by dinv_loc, AllGather -> bh_all [8, 2, 128, NB] f16."""
                st = state[b]
                bh_loc = dp.tile([2, 128, NB], F16, tag=f"agl_{b}_{round_id}")
                bh_all = dp.tile([NCORES, 2, 128, NB], F16,
                                 tag=f"aga_{b}_{round_id}")
                for fh in range(2):
                    hs = wp.tile([128, NB], F16, tag=f"hsc_{b}")
                    nc.vector.tensor_tensor(out=hs[:], in0=hT[fh][:],
                                            in1=st["dinv_rep16"][:], op=ALU.mult)
                    nc.sync.dma_start(bh_loc[fh, :, :], hs[:])
                nc.gpsimd.collective_compute(
                    "AllGather", ALU.bypass, ins=[bh_loc[:].opt()],
                    outs=[bh_all[:].opt()], replica_groups=RG)
                return bh_all

            def wmatmul(b, bh_all, Wnm):
                """X[:, t, :] = hT-slices @ W, streaming AG output per rank."""
                X = xp.tile([128, NT, FP], F16, tag=f"X_{b}")
                for k in range(NCORES):
                    hTk = hp.tile([128, 2, NB], F16, tag=f"hTk_{b}")
                    nc.scalar.dma_start(
                        hTk[:], bh_all[k].rearrange("fh f c -> f fh c"))
                    for j in range(4):
                        ps = psB.tile([128, FP], F32, tag="xps")
                        for kh in range(2):
                            nc.tensor.matmul(
                                ps[:], lhsT=hTk[:, kh, j * 128:(j + 1) * 128],
                                rhs=Wsl(Wnm, kh), start=(kh == 0), stop=(kh == 1))
                        nc.vector.tensor_copy(out=X[:, k * 4 + j, :], in_=ps[:])
                return X

            def lin_local(b, hT, Wnm, bias_nm, relu, out_dtype=F16):
                """yT[mh] = act(W^T @ hT + b), sharded [128, NB]."""
                out = []
                for mh in range(2):
                    ps = psA.tile([128, NB], F32, tag=f"msgps_{b}")
                    for kh in range(2):
                        nc.tensor.matmul(
                            ps[:], lhsT=Wsl(Wnm, kh, mh),
                            rhs=hT[kh][:], start=(kh == 0), stop=(kh == 1))
                    y = wp.tile([128, NB], out_dtype, tag=f"lin_{b}_{mh}")
                    if relu:
                        nc.vector.tensor_scalar(
                            out=y[:], in0=ps[:], scalar1=Bsl(bias_nm, mh),
                            scalar2=0.0, op0=ALU.add, op1=ALU.max)
                    else:
                        nc.vector.tensor_scalar_add(
                            out=y[:], in0=ps[:], scalar1=Bsl(bias_nm, mh))
                    out.append(y)
                return out

            # ================= branch computation (stage-interleaved) ======
            def stage_build(b):
                build_branch(b)
                st = state[b]
                if b == "t":
                    # topo feature: rowsum of raw w (incl +1 self loop)
                    rowsum = wp.tile([128, NT], F32, tag="rowsum")
                    nc.vector.reduce_sum(out=rowsum[:], in_=st["wperm"][:], axis=AX)
                    rs_loc = dp.tile([N, 1], F32, tag="rsloc")
                    rs_sum = dp.tile([N, 1], F32, tag="rssum")
                    nc.sync.dma_start(
                        rs_loc[:].rearrange("(t p) one -> p (t one)", p=128),
                        rowsum[:])
                    nc.gpsimd.collective_compute(
                        "AllReduce", ALU.add, ins=[rs_loc[:].opt()],
                        outs=[rs_sum[:].opt()], replica_groups=RG)
                    st["rs_sum"] = rs_sum

            def stage_g0(b):
                st = state[b]
                if b == "t":
                    rs_glob = wp.tile([128, NT], F32, tag="rsglob")
                    nc.scalar.dma_start(
                        rs_glob[:],
                        st["rs_sum"][:].rearrange("(t p) one -> p (t one)", p=128))
                    # tot = sum(rs_glob) ; stot = S_TOPO / (tot - 4096)
                    rsr = wp.tile([128, 1], F32, tag="rsr")
                    nc.vector.reduce_sum(out=rsr[:], in_=rs_glob[:], axis=AX)
                    tot_ps = psS.tile([1, 1], F32, tag="small")
                    nc.tensor.matmul(tot_ps[:], lhsT=rsr[:],
                                     rhs=onescol[:], start=True, stop=True)
                    stot = wp.tile([1, 1], F32, tag="stot")
                    nc.vector.tensor_scalar_add(out=stot[:], in0=tot_ps[:],
                                                scalar1=-4096.0)
                    nc.vector.reciprocal(stot[:], stot[:])
                    nc.vector.tensor_scalar_mul(out=stot[:], in0=stot[:],
                                                scalar1=S_TOPO)
                    # v = dinv_pcol * (rs_glob - 1)
                    v = wp.tile([128, NT], F32, tag="vfeat")
                    nc.vector.tensor_scalar_add(out=v[:], in0=rs_glob[:],
                                                scalar1=-1.0)
                    nc.vector.tensor_tensor(out=v[:], in0=v[:],
                                            in1=st["dinv_pcol"][:], op=ALU.mult)
                    v16 = wp.tile([128, NT], F16, tag="v16")
                    nc.vector.tensor_copy(out=v16[:], in_=v[:])
                    # u = v^T @ A -> [1, NB];  uu = u * dinv_loc * stot
                    u_ps = psS.tile([1, NB], F32, tag="small")
                    for t in range(NT):
                        nc.tensor.matmul(u_ps[:], lhsT=v16[:, t:t + 1],
                                         rhs=st["A"][t][:],
                                         start=(t == 0), stop=(t == NT - 1))
                    uu = wp.tile([1, NB], F32, tag="uu")
                    nc.vector.tensor_tensor(out=uu[:], in0=u_ps[:],
                                            in1=st["dinv_loc"][:], op=ALU.mult)
                    nc.vector.tensor_scalar_mul(out=uu[:], in0=uu[:],
                                                scalar1=stot[0:1, 0:1])
                    uu16 = wp.tile([1, NB], F16, tag="uu16")
                    nc.vector.tensor_copy(out=uu16[:], in_=uu[:])
                    # h0T[fh] = relu(tg0W[fh] (x) uu + b)
                    hT = []
                    for fh in range(2):
                        ps = psA.tile([128, NB], F32, tag=f"msgps_{b}")
                        nc.tensor.matmul(
                            ps[:], lhsT=tg0W16[:, fh * 128:(fh + 1) * 128],
                            rhs=uu16[:], start=True, stop=True)
                        h = wp.tile([128, NB], F16, tag=f"hT_{b}_{fh}")
                        nc.vector.tensor_scalar(
                            out=h[:], in0=ps[:],
                            scalar1=Bsl("tg0b", fh), scalar2=0.0,
                            op0=ALU.add, op1=ALU.max)
                        hT.append(h)
                else:
                    # traffic g0: X0 = dinv * W0 (W0 pre-scaled by S_TRAF)
                    X = xp.tile([128, NT, FP], F16, tag=f"X_{b}")
                    nc.scalar.dma_start(
                        X[:], dram_in["fg0W16"].ap().rearrange(
                            "(t p) n -> p t n", p=128))
                    for t in range(NT):
                        nc.vector.tensor_scalar_mul(
                            out=X[:, t, :], in0=X[:, t, :],
                            scalar1=st["dinv_pcol"][:, t:t + 1])
                    hT = conv_msg(b, X, "fg0b")
                st["hT"] = hT

            def stage_conv(b, g, lin_before=None):
                st = state[b]
                hT = st["hT"]
                if lin_before:
                    hT = lin_local(b, hT, f"{b}{lin_before}W16",
                                   f"{b}{lin_before}b", True)
                bh_all = allgather_h(b, hT, g)
                X = wmatmul(b, bh_all, f"{b}{g}W16")
                hT = conv_msg(b, X, f"{b}{g}b")
                st["hT"] = hT

            def stage_tail(b):
                st = state[b]
                hT = lin_local(b, st["hT"], f"{b}l1W16", f"{b}l1b", True)
                yT = lin_local(b, hT, f"{b}l2W16", f"{b}l2b", False,
                               out_dtype=F32)
                S = S_TOPO if b == "t" else S_TRAF
                mp = []
                for fh in range(2):
                    m = wp.tile([128, 1], F32, tag=f"mean_{b}_{fh}")
                    nc.vector.reduce_sum(out=m[:], in_=yT[fh][:], axis=AX)
                    nc.vector.tensor_scalar_mul(out=m[:], in0=m[:],
                                                scalar1=1.0 / (4096.0 * S))
                    mp.append(m)
                return mp

            stage_build("t")
            stage_build("f")
            stage_g0("t")
            stage_g0("f")
            for b in ("t", "f"):
                stage_conv(b, "g1")
            for b in ("t", "f"):
                stage_conv(b, "g2", lin_before="l0")
            for b in ("t", "f"):
                stage_conv(b, "g3")
            mp_t = stage_tail("t")
            mp_f = stage_tail("f")

            # ---------------- final AllReduce of branch means ----------------
            fin_loc = dp.tile([512, 1], F32, tag="finloc")
            fin_sum = dp.tile([512, 1], F32, tag="finsum")
            for j, m in enumerate(mp_t + mp_f):
                nc.scalar.dma_start(fin_loc[j * 128:(j + 1) * 128, :], m[:])
            nc.gpsimd.collective_compute(
                "AllReduce", ALU.add, ins=[fin_loc[:].opt()],
                outs=[fin_sum[:].opt()], replica_groups=RG)
            fin_sb = wp.tile([128, 4], F32, tag="finsb")
            nc.scalar.dma_start(
                fin_sb[:], fin_sum[:].rearrange("(q p) one -> p (q one)", p=128))

            # cat layout [128, 6]: fin(4 chunks) + rvec(2 chunks)
            cat = wp.tile([128, 6], F32, tag="cat")
            nc.vector.tensor_copy(out=cat[:, 0:4], in_=fin_sb[:])
            nc.vector.tensor_copy(out=cat[:, 4:6], in_=rvec[:])

            # head j0 (col-sharded): y1_blk = relu(cat @ j0Ws_k + j0b_k) [1,128]
            ps = psS.tile([1, 128], F32, tag="small")
            for q in range(6):
                nc.tensor.matmul(
                    ps[:], lhsT=cat[:, q:q + 1],
                    rhs=j0Ws[:, q, :],
                    start=(q == 0), stop=(q == 5))
            y1b = wp.tile([1, 128], F32, tag="y1b")
            nc.vector.tensor_tensor(out=y1b[:], in0=ps[:], in1=j0bk[:], op=ALU.add)
            nc.vector.tensor_scalar_max(out=y1b[:], in0=y1b[:], scalar1=0.0)
            # transpose y1_blk via DRAM bounce -> [128, 1]
            y1d = dp.tile([128, 1], F32, tag="y1d")
            nc.scalar.dma_start(y1d[:].rearrange("a one -> one a"), y1b[:])
            y1col = wp.tile([128, 1], F32, tag="y1col")
            nc.scalar.dma_start(y1col[:], y1d[:])
            # j1 partial (row-sharded): part = y1_blk @ j1W_k  [1, 512]
            ps = psS.tile([1, 512], F32, tag="small")
            nc.tensor.matmul(ps[:], lhsT=y1col[:],
                             rhs=j1Wk[:], start=True, stop=True)
            y2p = wp.tile([1, 512], F32, tag="y2p")
            nc.vector.tensor_copy(out=y2p[:], in_=ps[:])
            h_loc = dp.tile([1, 512], F32, tag="hloc")
            h_sum = dp.tile([1, 512], F32, tag="hsum")
            nc.scalar.dma_start(h_loc[:], y2p[:])
            nc.gpsimd.collective_compute(
                "AllReduce", ALU.add, ins=[h_loc[:].opt()],
                outs=[h_sum[:].opt()], replica_groups=RG)
            yf = wp.tile([1, 512], F32, tag="yf")
            nc.scalar.dma_start(yf[:], h_sum[:])
            nc.vector.tensor_tensor(out=yf[:], in0=yf[:], in1=j1b[:], op=ALU.add)
            nc.sync.dma_start(out_dram[:, :], yf[:])

    nc.compile()
    return nc


_NC = None


def _get_nc():
    global _NC
    if _NC is None:
        _NC = _build_kernel()
    return _NC


def _make_in_maps(topo_edge_index, topo_edge_weight, traffic_edge_index,
                  traffic_edge_weight, routing, params):
    prep_t = _prep_branch(topo_edge_index, topo_edge_weight)
    prep_f = _prep_branch(traffic_edge_index, traffic_edge_weight)
    shared = _prep_weights(params, routing)
    in_maps = []
    for k in range(NCORES):
        m = {kk: vv for kk, vv in shared.items()
             if kk not in ("j0Ws_k", "j0b_k", "j1W_k")}
        m["j0Ws_k"] = shared["j0Ws_k"][k]
        m["j0b_k"] = shared["j0b_k"][k]
        m["j1W_k"] = shared["j1W_k"][k]
        for b, prep in (("t", prep_t), ("f", prep_f)):
            m[f"{b}_wperm"] = prep[k]["wperm"]
            m[f"{b}_idx"] = prep[k]["idx"]
            m[f"{b}_m1"] = prep[k]["m1"]
            m[f"{b}_m2"] = prep[k]["m2"]
            m[f"{b}_m3"] = prep[k]["m3"]
        in_maps.append(m)
    return in_maps


def run(inputs, trace=False, trace_kwargs=None):
    nc = _get_nc()
    in_maps = _make_in_maps(**inputs)
    res = run_bass_kernel_spmd(
        nc, in_maps, core_ids=list(range(NCORES)), trace=trace,
        trace_kwargs=trace_kwargs or {})
    out = res.results[0]["out"].reshape(512).astype(np.float32)
    return out, res


def kernel(topo_edge_index, topo_edge_weight, traffic_edge_index,
           traffic_edge_weight, routing, params):
    out, _ = run(dict(topo_edge_index=topo_edge_index,
                      topo_edge_weight=topo_edge_weight,
                      traffic_edge_index=traffic_edge_index,
                      traffic_edge_weight=traffic_edge_weight,
                      routing=routing, params=params))
    return out
